# revision 1
# baseline (speedup 1.0000x reference)
"""ExpertsChooseMlp Trainium2 kernel.

Full inputs in, full output out. Sharding: 8 cores = 4 batches x 2 expert-pairs.
Core m handles batch b=m//2 and experts {2g, 2g+1}, g=m%2. Each core computes
pout[T,O] = sum_{e in pair} combine[b,:,e,:] @ mlp_e(dispatch[b,:,e,:]^T @ x[b]);
the host sums the two partials per batch and adds b2.

All matmuls run in bf16 with fp32 PSUM accumulation. Layouts are chosen so the
natural (host-prepared) operand orientations feed the PE directly:
  xdT[D,C] = matmul(lhsT=x[b][T,D],  rhs=dm_e[T,C])     (K=T)
  hT[HE,C] = matmul(lhsT=w1_e[D,HE], rhs=xdT[D,C])      (K=D), then GELU+b1
  y[C,O]   = matmul(lhsT=hT[HE,C],   rhs=w2_e[HE,O])    (K=HE)
  pout[T,O]= matmul(lhsT=cmT_e[C,T], rhs=y[C,O])        (K=C, accum over e)
Only cmT (combine slice transposed) is materialized host-side.
"""
import sys

sys.path.insert(0, "/opt/trn_rl_repo")

import numpy as np
import ml_dtypes

import concourse.bacc as bacc
import concourse.mybir as mybir
import concourse.tile as tile
from concourse import bass_utils

B, T, D, E, C, HE, O = 4, 2048, 512, 4, 1024, 512, 512
P = 128
nKT = T // P      # 16 T-chunks
nMD = D // P      # 4  D-chunks
nMH = HE // P     # 4  HE-chunks
nKD = D // P      # 4
nCC = C // P      # 8  C-chunks
nKH = HE // P     # 4
nMT = T // P      # 16
NF = 512          # matmul free dim (one PSUM bank)

F32 = mybir.dt.float32
BF16 = mybir.dt.bfloat16
GELU = mybir.ActivationFunctionType.Gelu

_NC = None


def _build():
    nc = bacc.Bacc("TRN2", target_bir_lowering=False, debug=False,
                   enable_asserts=False, num_devices=8)
    xb = nc.dram_tensor("xb", [T, D], BF16, kind="ExternalInput").ap()
    dm = nc.dram_tensor("dm", [2, T, C], BF16, kind="ExternalInput").ap()
    cmt = nc.dram_tensor("cmt", [2, C, T], BF16, kind="ExternalInput").ap()
    w1 = nc.dram_tensor("w1", [2, D, HE], BF16, kind="ExternalInput").ap()
    w2 = nc.dram_tensor("w2", [2, HE, O], BF16, kind="ExternalInput").ap()
    b1 = nc.dram_tensor("b1s", [2, HE], F32, kind="ExternalInput").ap()
    pout = nc.dram_tensor("pout", [T, O], F32, kind="ExternalOutput").ap()

    with tile.TileContext(nc) as tc:
        with (
            tc.tile_pool(name="const", bufs=1) as const,
            tc.tile_pool(name="dmp", bufs=18) as dmp,
            tc.tile_pool(name="cmp", bufs=16) as cmp_,
            tc.tile_pool(name="inter", bufs=2) as inter,
            tc.tile_pool(name="yp", bufs=2) as yp,
            tc.tile_pool(name="outp", bufs=4) as outp,
            tc.tile_pool(name="psum", bufs=4, space="PSUM") as psp,
        ):
            # ---- resident constants (ACT HWDGE ring) ----
            x_sb = const.tile([P, nKT, D], BF16)
            nc.scalar.dma_start(x_sb[:], xb.rearrange("(kt p) j -> p kt j", p=P))
            w1_sb = const.tile([P, 2, nKD, HE], BF16)
            nc.scalar.dma_start(w1_sb[:], w1.rearrange("e (kd p) j -> p e kd j", p=P))
            w2_sb = const.tile([P, 2, nKH, O], BF16)
            nc.scalar.dma_start(w2_sb[:], w2.rearrange("e (kh p) j -> p e kh j", p=P))
            b1_sb = const.tile([P, 2 * nMH], F32)
            nc.scalar.dma_start(b1_sb[:], b1.rearrange("e (mh p) -> p (e mh)", p=P))

            # ---- combine-mask tiles: dedicated slots, prefetch all (ACT ring) ----
            cmt_t = {}
            for ei in range(2):
                for kc in range(nCC):
                    t_ = cmp_.tile([P, T], BF16, tag="cmt")
                    nc.scalar.dma_start(t_[:], cmt[ei, kc * P:(kc + 1) * P, :])
                    cmt_t[(ei, kc)] = t_

            y_tiles = []
            for ei in range(2):
                # ---- dispatch-mask tiles for this expert (SYNC ring) ----
                dm_t = []
                for kt in range(nKT):
                    t_ = dmp.tile([P, C], BF16, tag="dm")
                    nc.sync.dma_start(t_[:], dm[ei, kt * P:(kt + 1) * P, :])
                    dm_t.append(t_)

                # ---- phase A: xdT[D, C] ----
                xdt = inter.tile([P, nMD, C], BF16, tag="xdt")
                for mc in range(nMD):
                    ps0 = psp.tile([P, NF], F32, tag="ps")
                    ps1 = psp.tile([P, NF], F32, tag="ps")
                    for kt in range(nKT):
                        lhsT = x_sb[:, kt, mc * P:(mc + 1) * P]
                        nc.tensor.matmul(ps0[:], lhsT, dm_t[kt][:, 0:NF],
                                         start=(kt == 0), stop=(kt == nKT - 1))
                        nc.tensor.matmul(ps1[:], lhsT, dm_t[kt][:, NF:C],
                                         start=(kt == 0), stop=(kt == nKT - 1))
                    nc.vector.tensor_copy(xdt[:, mc, 0:NF], ps0[:])
                    nc.vector.tensor_copy(xdt[:, mc, NF:C], ps1[:])

                # ---- phase B: hT[HE, C] = gelu(w1^T xdT + b1) ----
                ht = inter.tile([P, nMH, C], BF16, tag="ht")
                for mh in range(nMH):
                    ps0 = psp.tile([P, NF], F32, tag="ps")
                    ps1 = psp.tile([P, NF], F32, tag="ps")
                    for kd in range(nKD):
                        lhsT = w1_sb[:, ei, kd, mh * P:(mh + 1) * P]
                        nc.tensor.matmul(ps0[:], lhsT, xdt[:, kd, 0:NF],
                                         start=(kd == 0), stop=(kd == nKD - 1))
                        nc.tensor.matmul(ps1[:], lhsT, xdt[:, kd, NF:C],
                                         start=(kd == 0), stop=(kd == nKD - 1))
                    bia = b1_sb[:, ei * nMH + mh:ei * nMH + mh + 1]
                    nc.scalar.activation(ht[:, mh, 0:NF], ps0[:], GELU, bias=bia)
                    nc.scalar.activation(ht[:, mh, NF:C], ps1[:], GELU, bias=bia)

                # ---- phase C: y[C, O] ----
                y_sb = yp.tile([P, nCC, O], BF16, tag="y")
                for cc in range(nCC):
                    ps = psp.tile([P, NF], F32, tag="ps")
                    for kh in range(nKH):
                        nc.tensor.matmul(ps[:], ht[:, kh, cc * P:(cc + 1) * P],
                                         w2_sb[:, ei, kh, :],
                                         start=(kh == 0), stop=(kh == nKH - 1))
                    nc.vector.tensor_copy(y_sb[:, cc, :], ps[:])
                y_tiles.append(y_sb)

            # ---- phase D: pout[T, O] = sum_e cmT_e^T y_e ----
            for mt in range(nMT):
                ps = psp.tile([P, NF], F32, tag="ps")
                idx = 0
                for ei in range(2):
                    for kc in range(nCC):
                        nc.tensor.matmul(ps[:],
                                         cmt_t[(ei, kc)][:, mt * P:(mt + 1) * P],
                                         y_tiles[ei][:, kc, :],
                                         start=(idx == 0), stop=(idx == 15))
                        idx += 1
                ot = outp.tile([P, O], F32, tag="out")
                nc.vector.tensor_copy(ot[:], ps[:])
                nc.sync.dma_start(pout[mt * P:(mt + 1) * P, :], ot[:])

    nc.compile()
    return nc


def get_nc():
    global _NC
    if _NC is None:
        _NC = _build()
    return _NC


def make_in_maps(x, dispatch_mask, combine_array, w1, b1, w2):
    bf = ml_dtypes.bfloat16
    in_maps = []
    for m in range(8):
        b, g = m // 2, m % 2
        es = slice(2 * g, 2 * g + 2)
        dm_s = np.ascontiguousarray(
            np.transpose(dispatch_mask[b, :, es, :], (1, 0, 2))).astype(bf)
        cmt_s = np.ascontiguousarray(
            np.transpose(combine_array[b, :, es, :], (1, 2, 0))).astype(bf)
        in_maps.append({
            "xb": np.ascontiguousarray(x[b]).astype(bf),
            "dm": dm_s,
            "cmt": cmt_s,
            "w1": np.ascontiguousarray(w1[es]).astype(bf),
            "w2": np.ascontiguousarray(w2[es]).astype(bf),
            "b1s": np.ascontiguousarray(b1[es]).astype(np.float32),
        })
    return in_maps


def kernel(x, dispatch_mask, combine_array, w1, b1, w2, b2):
    nc = get_nc()
    in_maps = make_in_maps(x, dispatch_mask, combine_array, w1, b1, w2)
    res = bass_utils.run_bass_kernel_spmd(nc, in_maps, core_ids=list(range(8)))
    b2f = np.asarray(b2, dtype=np.float32)
    out = np.empty((B, T, O), dtype=np.float32)
    for b in range(B):
        out[b] = res.results[2 * b]["pout"] + res.results[2 * b + 1]["pout"] + b2f
    return out
